# revision 13
# baseline (speedup 1.0000x reference)
"""Two-layer GAT (DGL GATConv + DenseNet buffer skips) on 8 Trainium2 NeuronCores.

Strategy (self-contained, hardcoded for N=20000, E=320000, 256-dim, 8+1 heads):
- Nodes sharded 2500/core (padded to 2560); edges partitioned by dst owner.
- Per core, edges grouped into 20 windows of 128 local dst nodes, chunks of 128
  edges, chunk count per window padded to the global max C (SPMD-uniform).
- Per-layer feature tables ([feat | el | pad] bf16 rows, 768B) built by sharded
  GEMMs (20 local node tiles) and AllGathered into per-core HBM.
- Per-edge source rows fetched with gpsimd.dma_gather (1024 idx/call).
- Edge softmax without max-subtraction (logits are O(1)): ex = exp(lrelu(el+er));
  er expanded per-edge via a one-hot^T matmul from resident local er.
- Scatter-sum as TensorE one-hot matmuls accumulating [128 dst, 256+H] PSUM
  tiles (trailing cols accumulate ssum); normalize on drain.
- Pad edges carry all-zero one-hot columns, so they contribute exactly nothing.
- h1 kept only transposed (PE transpose per window) as GEMM2 stationary.
"""

import sys
import numpy as np
import ml_dtypes

sys.path.insert(0, "/opt/trn_rl_repo")

import concourse.bacc as bacc
import concourse.bass as bass
import concourse.tile as tile
import concourse.mybir as mybir
from concourse.bass_utils import run_bass_kernel_spmd

bf16 = ml_dtypes.bfloat16
FP32 = mybir.dt.float32
BF16 = mybir.dt.bfloat16
I16 = mybir.dt.int16

N, E, M = 20000, 320000, 8
NL, NLP = 2500, 2560          # local nodes, padded
NP = NLP * M                  # 20480 padded global rows
TW = 384                      # table row width (bf16) = 768B
LT = NLP // 128               # 20 local node tiles = windows per core
H1, D1 = 8, 32

_cache = {}


def _build(Cw):
    """Build the SPMD Bass program; Cw[w] = chunks in window w (max over cores)."""
    NCH = sum(Cw)             # chunks per core
    EPAD = NCH * 128
    IW = EPAD // 16           # idx tile cols
    nbatch = (NCH + 7) // 8
    # chunk -> (window, chunk-in-window, is-last)
    cmap = []
    for w, c in enumerate(Cw):
        for j in range(c):
            cmap.append((w, j == 0, j == c - 1))

    nc = bacc.Bacc("TRN2", debug=False)
    dt = nc.dram_tensor
    xtl_d = dt("xtl", [LT, 2, 128, 128], BF16, kind="ExternalInput")  # local x^T tiles
    xtt_d = dt("xtt", [80, 128, 2, 2, 128], BF16, kind="ExternalInput")  # full x^T tile-pairs
    idx = dt("idx", [128, IW], I16, kind="ExternalInput")
    oh_d = dt("oh", [NCH, 128, 128], BF16, kind="ExternalInput")
    oht_d = dt("oht", [NCH, 128, 128], BF16, kind="ExternalInput")
    w1_d = dt("w1", [128, 2, 264], BF16, kind="ExternalInput")     # [fc1T | Wl1]
    wb1_d = dt("wb1c", [128, 2, 264], BF16, kind="ExternalInput")  # [wb0T | Wr1]
    w2_d = dt("w2", [128, 2, 257], BF16, kind="ExternalInput")     # [fc2T | Wl2]
    wb2_d = dt("wb2c", [128, 4, 257], BF16, kind="ExternalInput")  # [wb1T | 0;Wr2]
    b1_d = dt("b1r", [128, 256], FP32, kind="ExternalInput")
    bb0_d = dt("bb0r", [128, 256], FP32, kind="ExternalInput")
    b2_d = dt("b2r", [128, 256], FP32, kind="ExternalInput")
    bb1_d = dt("bb1r", [128, 256], FP32, kind="ExternalInput")
    nw_d = dt("normw", [128, LT], FP32, kind="ExternalInput")
    id_d = dt("ident", [128, 128], BF16, kind="ExternalInput")
    out_d = dt("out", [NLP, 256], FP32, kind="ExternalOutput")

    with tile.TileContext(nc) as tc:
        with (
            tc.tile_pool(name="const", bufs=1) as cp,
            tc.tile_pool(name="stat", bufs=4) as statp,
            tc.tile_pool(name="stage", bufs=6) as stgp,
            tc.tile_pool(name="gat", bufs=5) as gatp,
            tc.tile_pool(name="ohp", bufs=5) as ohp,
            tc.tile_pool(name="ohtp", bufs=5) as ohtp,
            tc.tile_pool(name="edge", bufs=5) as edp,
            tc.tile_pool(name="wep", bufs=2) as wep,
            tc.tile_pool(name="pg", bufs=2, space="PSUM") as pg,
            tc.tile_pool(name="pw", bufs=3, space="PSUM") as pw,
            tc.tile_pool(name="per", bufs=2, space="PSUM") as per,
            tc.tile_pool(name="pt", bufs=1, space="PSUM") as pt,
            tc.tile_pool(name="dram", bufs=1, space="DRAM") as dp,
        ):
            # ---- residents ----
            idx_sb = cp.tile([128, IW], I16)
            nc.sync.dma_start(idx_sb[:], idx[:])
            xtl = cp.tile([128, LT, 2, 128], BF16)   # [p, tile, k, f]
            nc.sync.dma_start(xtl[:], xtl_d[:].rearrange("t k p f -> p t k f"))
            w1 = cp.tile([128, 2, 264], BF16); nc.sync.dma_start(w1[:], w1_d[:])
            wb1 = cp.tile([128, 2, 264], BF16); nc.sync.dma_start(wb1[:], wb1_d[:])
            w2 = cp.tile([128, 2, 257], BF16); nc.sync.dma_start(w2[:], w2_d[:])
            wb2 = cp.tile([128, 4, 257], BF16); nc.sync.dma_start(wb2[:], wb2_d[:])
            b1r = cp.tile([128, 256], FP32); nc.sync.dma_start(b1r[:], b1_d[:])
            bb0r = cp.tile([128, 256], FP32); nc.sync.dma_start(bb0r[:], bb0_d[:])
            b2r = cp.tile([128, 256], FP32); nc.sync.dma_start(b2r[:], b2_d[:])
            bb1r = cp.tile([128, 256], FP32); nc.sync.dma_start(bb1r[:], bb1_d[:])
            nw = cp.tile([128, LT], FP32); nc.sync.dma_start(nw[:], nw_d[:])
            ident = cp.tile([128, 128], BF16); nc.sync.dma_start(ident[:], id_d[:])
            buf1 = cp.tile([128, LT, 256], FP32)
            er1 = cp.tile([128, LT, 8], BF16)
            buf2 = cp.tile([128, LT, 256], FP32)
            er2 = cp.tile([128, LT, 1], BF16)
            h1T = cp.tile([128, 2, NLP], BF16)

            table1 = dp.tile([NP, TW], BF16)
            table2 = dp.tile([4, M, 5 * 128, TW], BF16)
            sh2 = dp.tile([NLP, TW], BF16)    # local table2 shard pre-allgather

            def gemm_local(stats, mov, ncols, shard, extra):
                for i in range(LT):
                    ps = pg.tile([128, ncols], FP32, tag="ps_g")
                    for k in range(len(mov)):
                        nc.tensor.matmul(ps[:], stats(i, k), mov[k],
                                         start=(k == 0), stop=(k == len(mov) - 1))
                    if shard is not None:
                        stg = stgp.tile([128, ncols], BF16, tag="stg")
                        nc.scalar.copy(stg[:], ps[:])
                        nc.sync.dma_start(shard[i * 128:(i + 1) * 128, 0:ncols], stg[:])
                    if extra is not None:
                        extra(i, ps)

            def stat_xl(i, k):
                return xtl[:, i, k, :]

            # ---- GEMM1 (replicated): table1 = [fc1(x) | el1] for all nodes ----
            for p in range(80):
                ld = statp.tile([128, 2, 2, 128], BF16, tag="ld")
                nc.sync.dma_start(ld[:], xtt_d[p])
                for j in range(2):
                    nt = 2 * p + j
                    ci, li = nt // LT, nt % LT
                    row0 = (li // 5) * (M * 640) + ci * 640 + (li % 5) * 128
                    ps = pg.tile([128, 264], FP32, tag="ps_g")
                    nc.tensor.matmul(ps[:], ld[:, j, 0, :], w1[:, 0, :], start=True, stop=False)
                    nc.tensor.matmul(ps[:], ld[:, j, 1, :], w1[:, 1, :], start=False, stop=True)
                    stg = stgp.tile([128, 264], BF16, tag="stg")
                    if j == 0:
                        nc.scalar.copy(stg[:], ps[:])
                    else:
                        nc.vector.tensor_copy(stg[:], ps[:])
                    nc.sync.dma_start(table1[row0:row0 + 128, 0:264], stg[:])

            # ---- local buffer GEMM1: buf1 = x@wb0.T + bb0, er1 local ----
            def keep1(i, ps):
                nc.vector.tensor_tensor(buf1[:, i, :], ps[:, 0:256], bb0r[:],
                                        mybir.AluOpType.add)
                nc.vector.tensor_copy(er1[:, i, :], ps[:, 256:264])
            gemm_local(stat_xl, [wb1[:, 0, :], wb1[:, 1, :]], 264, None, keep1)

            # ---- edge phase ----
            def edge_phase(table, er_loc, heads, drain):
                segw = 256 + heads
                wtiles = {}
                for b in range(nbatch):
                    c0 = b * 8
                    sz = min(8, NCH - c0)
                    ni = sz * 128
                    g = gatp.tile([128, 8, TW], BF16, tag="g")
                    nc.gpsimd.dma_gather(g[:, 0:sz, :], table,
                                         idx_sb[:, c0 * 8:c0 * 8 + sz * 8], ni, ni, TW)
                    oh = ohp.tile([128, 8, 128], BF16, tag="oh")
                    nc.sync.dma_start(oh[:, 0:sz, :],
                                      oh_d[c0:c0 + sz].rearrange("c p f -> p c f"))
                    oht = ohtp.tile([128, 8, 128], BF16, tag="oht")
                    nc.scalar.dma_start(oht[:, 0:sz, :],
                                        oht_d[c0:c0 + sz].rearrange("c p f -> p c f"))
                    # er expansion: per chunk matmul oht^T @ er_w -> [128 e, heads]
                    pse = per.tile([128, 8 * heads], FP32, tag="pse")
                    for c in range(sz):
                        wv = cmap[c0 + c][0]
                        nc.tensor.matmul(pse[:, c * heads:(c + 1) * heads],
                                         oht[:, c, :], er_loc[:, wv, :],
                                         start=True, stop=True)
                    # ex = exp(max(0.2 z, z)), z = el + er
                    z = edp.tile([128, 8, heads], FP32, tag="z")
                    nc.vector.tensor_tensor(
                        z[:, 0:sz, :], g[:, 0:sz, 256:256 + heads],
                        pse[:].rearrange("p (c h) -> p c h", h=heads)[:, 0:sz, :],
                        mybir.AluOpType.add)
                    lr = edp.tile([128, 8, heads], FP32, tag="lr")
                    nc.vector.scalar_tensor_tensor(lr[:, 0:sz, :], z[:, 0:sz, :], 0.2,
                                                   z[:, 0:sz, :],
                                                   mybir.AluOpType.mult, mybir.AluOpType.max)
                    ex = edp.tile([128, 8, heads], BF16, tag="ex")
                    nc.scalar.activation(ex[:, 0:sz, :], lr[:, 0:sz, :],
                                         mybir.ActivationFunctionType.Exp)
                    # scaled = [feat * ex (per head) | ex]
                    sc = edp.tile([128, 8, segw], BF16, tag="sc")
                    if heads == 8:
                        nc.vector.tensor_tensor(
                            sc[:, 0:sz, 0:256].rearrange("p c (h j) -> p c h j", h=8),
                            g[:, 0:sz, 0:256].rearrange("p c (h j) -> p c h j", h=8),
                            ex[:, 0:sz, :].unsqueeze(3).to_broadcast((128, sz, 8, 32)),
                            mybir.AluOpType.mult)
                    else:
                        nc.vector.tensor_tensor(
                            sc[:, 0:sz, 0:256], g[:, 0:sz, 0:256],
                            ex[:, 0:sz, :].to_broadcast((128, sz, 256)),
                            mybir.AluOpType.mult)
                    nc.vector.tensor_copy(sc[:, 0:sz, 256:segw], ex[:, 0:sz, :])
                    # windowed scatter-sum
                    for c in range(sz):
                        wv, first, last = cmap[c0 + c]
                        if first:
                            psw_t = pw.tile([128, segw], FP32, tag="psw")
                            wtiles[wv] = psw_t
                        nc.tensor.matmul(wtiles[wv][:], oh[:, c, :], sc[:, c, :],
                                         start=first, stop=last)
                        if last:
                            drain(wv, wtiles.pop(wv))

            # ---- layer-1 window drain: h1 -> h1T, then GEMM2a tile ----
            def drain1(wv, ps):
                sm = wep.tile([128, 8], FP32, tag="sm")
                nc.vector.tensor_scalar(sm[:], ps[:, 256:264], 1e-30, None,
                                        mybir.AluOpType.max)
                rec = wep.tile([128, 8], FP32, tag="rec")
                nc.vector.reciprocal(rec[:], sm[:])
                rn = wep.tile([128, 256], FP32, tag="rn")
                nc.vector.tensor_tensor(
                    rn[:].rearrange("p (h j) -> p h j", h=8),
                    ps[:, 0:256].rearrange("p (h j) -> p h j", h=8),
                    rec[:].unsqueeze(2).to_broadcast((128, 8, 32)),
                    mybir.AluOpType.mult)
                nc.vector.tensor_tensor(rn[:], rn[:], b1r[:], mybir.AluOpType.add)
                hpre = wep.tile([128, 256], FP32, tag="hpre")
                nc.vector.scalar_tensor_tensor(hpre[:], buf1[:, wv, :], nw[:, wv:wv + 1],
                                               rn[:], mybir.AluOpType.mult,
                                               mybir.AluOpType.add)
                h1b = wep.tile([128, 256], BF16, tag="h1b")
                nc.scalar.activation(h1b[:], hpre[:], mybir.ActivationFunctionType.Relu)
                for fh in range(2):
                    ptt = pt.tile([128, 128], BF16, tag="ptt")
                    nc.tensor.transpose(ptt[:], h1b[:, fh * 128:(fh + 1) * 128], ident[:])
                    nc.vector.tensor_copy(h1T[:, fh, wv * 128:(wv + 1) * 128], ptt[:])
                # GEMM2a for this tile: [fc2(h1) | el2] -> shard2
                ps2 = pg.tile([128, 257], FP32, tag="ps_g")
                for k in range(2):
                    nc.tensor.matmul(ps2[:], h1T[:, k, wv * 128:(wv + 1) * 128],
                                     w2[:, k, :], start=(k == 0), stop=(k == 1))
                stg = stgp.tile([128, 257], BF16, tag="stg")
                nc.scalar.copy(stg[:], ps2[:])
                nc.sync.dma_start(sh2[wv * 128:(wv + 1) * 128, 0:257], stg[:])
                # GEMM2b for this tile: buf2 = cat(x,h1)@wb1.T + bb1, er2 local
                ps3 = pg.tile([128, 257], FP32, tag="ps_g")
                for k in range(4):
                    st = stat_xl(wv, k) if k < 2 else h1T[:, k - 2, wv * 128:(wv + 1) * 128]
                    nc.tensor.matmul(ps3[:], st, wb2[:, k, :],
                                     start=(k == 0), stop=(k == 3))
                nc.vector.tensor_tensor(buf2[:, wv, :], ps3[:, 0:256], bb1r[:],
                                        mybir.AluOpType.add)
                nc.vector.tensor_copy(er2[:, wv, :], ps3[:, 256:257])

            edge_phase(table1[:], er1, 8, drain1)

            # ---- allgather table2 (grouped) ----
            for gg in range(4):
                nc.gpsimd.collective_compute(
                    "AllGather", mybir.AluOpType.bypass,
                    replica_groups=[list(range(M))],
                    ins=[sh2[gg * 640:(gg + 1) * 640, :]], outs=[table2[gg]])

            # ---- layer-2 window drain -> final output ----
            def drain2(wv, ps):
                sm = wep.tile([128, 1], FP32, tag="sm2")
                nc.vector.tensor_scalar(sm[:], ps[:, 256:257], 1e-30, None,
                                        mybir.AluOpType.max)
                rec = wep.tile([128, 1], FP32, tag="rec2")
                nc.vector.reciprocal(rec[:], sm[:])
                rst = wep.tile([128, 256], FP32, tag="rst2")
                nc.vector.tensor_scalar(rst[:], ps[:, 0:256], rec[:], None,
                                        mybir.AluOpType.mult)
                nc.vector.tensor_tensor(rst[:], rst[:], b2r[:], mybir.AluOpType.add)
                ow = wep.tile([128, 256], FP32, tag="ow")
                nc.vector.scalar_tensor_tensor(ow[:], buf2[:, wv, :], nw[:, wv:wv + 1],
                                               rst[:], mybir.AluOpType.mult,
                                               mybir.AluOpType.add)
                nc.sync.dma_start(out_d[wv * 128:(wv + 1) * 128, :], ow[:])

            edge_phase(table2[:].rearrange("g m r w -> (g m r) w"), er2, 1, drain2)

    nc.compile()
    return nc


def _wrap_idx(flat):
    """Per-batch (1024) column-major-of-16 wrap, tiled to 128 partitions."""
    out = []
    for s in range(0, len(flat), 1024):
        blk = flat[s:s + 1024]
        out.append(blk.reshape(-1, 16).T)
    return np.tile(np.concatenate(out, axis=1), (8, 1)).astype(np.int16)


def kernel(**inputs):
    x = np.asarray(inputs["inputs"], np.float32)
    src = np.asarray(inputs["src"], np.int64)
    dst = np.asarray(inputs["dst"], np.int64)

    deg = np.bincount(dst, minlength=N).astype(np.float32)
    norm = 1.0 / np.clip(deg, 1.0, None)
    src_l = src % NL
    src_c = src // NL
    src_n = src_c * NLP + src_l          # padded-global (for nothing but dst calc symmetry)
    dst_n = (dst // NL) * NLP + (dst % NL)
    # table rows are group-major: row(core k, local r) = (r//640)*5120 + k*640 + r%640
    src_t = (src_l // 640) * (M * 640) + src_c * 640 + (src_l % 640)

    # per-core edge lists grouped by window (128 local dst), padded to C chunks
    core_of = dst // NL
    dloc = dst_n % NLP
    win = dloc // 128
    counts = np.zeros((M, LT), np.int64)
    buckets = [[[] for _ in range(LT)] for _ in range(M)]
    order = np.argsort(core_of * LT + win, kind="stable")
    for e in order:
        k, w = core_of[e], win[e]
        buckets[k][w].append(e)
        counts[k, w] += 1
    Cw = tuple(int(np.ceil(counts[:, w].max() / 128)) for w in range(LT))
    NCH = sum(Cw)
    EPAD = NCH * 128
    wbase = np.concatenate([[0], np.cumsum([c * 128 for c in Cw])])

    if Cw not in _cache:
        _cache[Cw] = _build(Cw)
    nc = _cache[Cw]

    # fused weights
    fc1w = np.asarray(inputs["fc1_w"], np.float32)
    al1 = np.asarray(inputs["attn_l1"], np.float32)
    ar1 = np.asarray(inputs["attn_r1"], np.float32)
    Al = np.zeros((256, 8), np.float32); Ar = np.zeros((256, 8), np.float32)
    for h in range(H1):
        Al[h * D1:(h + 1) * D1, h] = al1[h]
        Ar[h * D1:(h + 1) * D1, h] = ar1[h]
    Wl1 = fc1w.T @ Al; Wr1 = fc1w.T @ Ar
    fc2w = np.asarray(inputs["fc2_w"], np.float32)
    Wl2 = fc2w.T @ np.asarray(inputs["attn_l2"], np.float32).T
    Wr2 = fc2w.T @ np.asarray(inputs["attn_r2"], np.float32).T
    w1c = np.concatenate([fc1w.T, Wl1], 1).astype(bf16)              # [256, 264]
    wb1c = np.concatenate([np.asarray(inputs["wb0"], np.float32).T, Wr1], 1).astype(bf16)
    w2c = np.concatenate([fc2w.T, Wl2], 1).astype(bf16)              # [256, 257]
    Wr2p = np.concatenate([np.zeros((256, 1), np.float32), Wr2], 0)
    wb2c = np.concatenate([np.asarray(inputs["wb1"], np.float32).T, Wr2p],
                          1).astype(bf16)                            # [512, 257]

    xp = np.zeros((NP, 256), np.float32)
    for k in range(M):
        xp[k * NLP:k * NLP + NL] = x[k * NL:(k + 1) * NL]
    xt_b = np.ascontiguousarray(xp.T).astype(bf16)                    # [256, NP]
    normp = np.zeros(NP, np.float32)
    for k in range(M):
        normp[k * NLP:k * NLP + NL] = norm[k * NL:(k + 1) * NL]

    xtt = np.zeros((80, 128, 2, 2, 128), bf16)
    for nt in range(160):
        p, j = nt // 2, nt % 2
        ci, li = nt // LT, nt % LT
        cols = slice(ci * NLP + li * 128, ci * NLP + (li + 1) * 128)
        xtt[p, :, j, 0, :] = xt_b[0:128, cols]
        xtt[p, :, j, 1, :] = xt_b[128:256, cols]

    def rep(v):
        return np.broadcast_to(np.asarray(v, np.float32)[None, :], (128, 256)).copy()

    common = {
        "w1": np.ascontiguousarray(w1c.reshape(2, 128, 264).transpose(1, 0, 2)),
        "wb1c": np.ascontiguousarray(wb1c.reshape(2, 128, 264).transpose(1, 0, 2)),
        "w2": np.ascontiguousarray(w2c.reshape(2, 128, 257).transpose(1, 0, 2)),
        "wb2c": np.ascontiguousarray(wb2c.reshape(4, 128, 257).transpose(1, 0, 2)),
        "b1r": rep(inputs["bias1"]), "bb0r": rep(inputs["bb0"]),
        "b2r": rep(inputs["bias2"]), "bb1r": rep(inputs["bb1"]),
        "ident": np.eye(128, dtype=bf16),
    }

    in_maps = []
    for k in range(M):
        gid = np.zeros(EPAD, np.int64)           # gather row ids (pads -> row 0)
        dl = np.zeros(EPAD, np.int64)
        real = np.zeros(EPAD, bool)
        for w in range(LT):
            es = buckets[k][w]
            base = int(wbase[w])
            gid[base:base + len(es)] = src_t[es]
            dl[base:base + len(es)] = dloc[es] - w * 128
            real[base:base + len(es)] = True
        oh = np.zeros((EPAD, 128), bf16)
        oh[np.arange(EPAD)[real], dl[real]] = bf16(1.0)
        oh = oh.reshape(NCH, 128, 128)
        oht = np.ascontiguousarray(oh.transpose(0, 2, 1))
        # local x^T tiles [LT, 2, 128, 128]
        xl = xt_b[:, k * NLP:(k + 1) * NLP]      # [256, 2560]
        xlt = np.ascontiguousarray(
            xl.reshape(2, 128, LT, 128).transpose(2, 0, 1, 3))
        im = dict(common)
        im["idx"] = _wrap_idx(gid)
        im["xtt"] = xtt
        im["oh"] = oh
        im["oht"] = oht
        im["xtl"] = xlt
        im["normw"] = np.ascontiguousarray(
            normp[k * NLP:(k + 1) * NLP].reshape(LT, 128).T)
        in_maps.append(im)

    res = run_bass_kernel_spmd(nc, in_maps, core_ids=list(range(M)), trace=False)
    out = np.empty((N, 256), np.float32)
    for k in range(M):
        out[k * NL:(k + 1) * NL] = res.results[k]["out"][0:NL]
    return out


# revision 14
# speedup vs baseline: 1.0691x; 1.0691x over previous
"""Two-layer GAT (DGL GATConv + DenseNet buffer skips) on 8 Trainium2 NeuronCores.

Strategy (self-contained, hardcoded for N=20000, E=320000, 256-dim, 8+1 heads):
- Nodes sharded 2500/core (padded to 2560); edges partitioned by dst owner.
- Per core, edges grouped into 20 windows of 128 local dst nodes, chunks of 128
  edges, chunk count per window padded to the global max C (SPMD-uniform).
- Per-layer feature tables ([feat | el | pad] bf16 rows, 768B) built by sharded
  GEMMs (20 local node tiles) and AllGathered into per-core HBM.
- Per-edge source rows fetched with gpsimd.dma_gather (1024 idx/call).
- Edge softmax without max-subtraction (logits are O(1)): ex = exp(lrelu(el+er));
  er expanded per-edge via a one-hot^T matmul from resident local er.
- Scatter-sum as TensorE one-hot matmuls accumulating [128 dst, 256+H] PSUM
  tiles (trailing cols accumulate ssum); normalize on drain.
- Pad edges carry all-zero one-hot columns, so they contribute exactly nothing.
- h1 kept only transposed (PE transpose per window) as GEMM2 stationary.
"""

import sys
import numpy as np
import ml_dtypes

sys.path.insert(0, "/opt/trn_rl_repo")

import concourse.bacc as bacc
import concourse.bass as bass
import concourse.tile as tile
import concourse.mybir as mybir
from concourse.bass_utils import run_bass_kernel_spmd

bf16 = ml_dtypes.bfloat16
FP32 = mybir.dt.float32
BF16 = mybir.dt.bfloat16
I16 = mybir.dt.int16

N, E, M = 20000, 320000, 8
NL, NLP = 2500, 2560          # local nodes, padded
NP = NLP * M                  # 20480 padded global rows
TW = 384                      # table row width (bf16) = 768B
LT = NLP // 128               # 20 local node tiles = windows per core
H1, D1 = 8, 32

_cache = {}


def _build(Cw):
    """Build the SPMD Bass program; Cw[w] = chunks in window w (max over cores)."""
    NCH = sum(Cw)             # chunks per core
    EPAD = NCH * 128
    IW = EPAD // 16           # idx tile cols
    nbatch = (NCH + 7) // 8
    # chunk -> (window, chunk-in-window, is-last)
    cmap = []
    for w, c in enumerate(Cw):
        for j in range(c):
            cmap.append((w, j == 0, j == c - 1))

    nc = bacc.Bacc("TRN2", debug=False)
    dt = nc.dram_tensor
    xtl_d = dt("xtl", [LT, 2, 128, 128], BF16, kind="ExternalInput")  # local x^T tiles
    xtt_d = dt("xtt", [80, 128, 2, 2, 128], BF16, kind="ExternalInput")  # full x^T tile-pairs
    idx = dt("idx", [128, IW], I16, kind="ExternalInput")
    oh_d = dt("oh", [NCH, 128, 128], BF16, kind="ExternalInput")
    oht_d = dt("oht", [NCH, 128, 128], BF16, kind="ExternalInput")
    w1_d = dt("w1", [128, 2, 264], BF16, kind="ExternalInput")     # [fc1T | Wl1]
    wb1_d = dt("wb1c", [128, 2, 264], BF16, kind="ExternalInput")  # [wb0T | Wr1]
    w2_d = dt("w2", [128, 2, 257], BF16, kind="ExternalInput")     # [fc2T | Wl2]
    wb2_d = dt("wb2c", [128, 4, 257], BF16, kind="ExternalInput")  # [wb1T | 0;Wr2]
    b1_d = dt("b1r", [128, 256], FP32, kind="ExternalInput")
    bb0_d = dt("bb0r", [128, 256], FP32, kind="ExternalInput")
    b2_d = dt("b2r", [128, 256], FP32, kind="ExternalInput")
    bb1_d = dt("bb1r", [128, 256], FP32, kind="ExternalInput")
    nw_d = dt("normw", [128, LT], FP32, kind="ExternalInput")
    id_d = dt("ident", [128, 128], BF16, kind="ExternalInput")
    out_d = dt("out", [NLP, 256], FP32, kind="ExternalOutput")

    with tile.TileContext(nc) as tc:
        with (
            tc.tile_pool(name="const", bufs=1) as cp,
            tc.tile_pool(name="stat", bufs=4) as statp,
            tc.tile_pool(name="stage", bufs=6) as stgp,
            tc.tile_pool(name="gat", bufs=5) as gatp,
            tc.tile_pool(name="ohp", bufs=5) as ohp,
            tc.tile_pool(name="ohtp", bufs=5) as ohtp,
            tc.tile_pool(name="edge", bufs=5) as edp,
            tc.tile_pool(name="wep", bufs=2) as wep,
            tc.tile_pool(name="pg", bufs=2, space="PSUM") as pg,
            tc.tile_pool(name="pw", bufs=3, space="PSUM") as pw,
            tc.tile_pool(name="per", bufs=2, space="PSUM") as per,
            tc.tile_pool(name="pt", bufs=1, space="PSUM") as pt,
            tc.tile_pool(name="dram", bufs=1, space="DRAM") as dp,
        ):
            # ---- residents ----
            idx_sb = cp.tile([128, IW], I16)
            nc.sync.dma_start(idx_sb[:], idx[:])
            xtl = cp.tile([128, LT, 2, 128], BF16)   # [p, tile, k, f]
            nc.sync.dma_start(xtl[:], xtl_d[:].rearrange("t k p f -> p t k f"))
            w1 = cp.tile([128, 2, 264], BF16); nc.sync.dma_start(w1[:], w1_d[:])
            wb1 = cp.tile([128, 2, 264], BF16); nc.sync.dma_start(wb1[:], wb1_d[:])
            w2 = cp.tile([128, 2, 257], BF16); nc.sync.dma_start(w2[:], w2_d[:])
            wb2 = cp.tile([128, 4, 257], BF16); nc.sync.dma_start(wb2[:], wb2_d[:])
            b1r = cp.tile([128, 256], FP32); nc.sync.dma_start(b1r[:], b1_d[:])
            bb0r = cp.tile([128, 256], FP32); nc.sync.dma_start(bb0r[:], bb0_d[:])
            b2r = cp.tile([128, 256], FP32); nc.sync.dma_start(b2r[:], b2_d[:])
            bb1r = cp.tile([128, 256], FP32); nc.sync.dma_start(bb1r[:], bb1_d[:])
            nw = cp.tile([128, LT], FP32); nc.sync.dma_start(nw[:], nw_d[:])
            ident = cp.tile([128, 128], BF16); nc.sync.dma_start(ident[:], id_d[:])
            buf1 = cp.tile([128, LT, 256], FP32)
            er1 = cp.tile([128, LT, 8], BF16)
            buf2 = cp.tile([128, LT, 256], FP32)
            er2 = cp.tile([128, LT, 1], BF16)
            h1T = cp.tile([128, 2, NLP], BF16)

            table1 = dp.tile([NP, TW], BF16)
            table2 = dp.tile([4, M, 5 * 128, TW], BF16)
            sh2 = dp.tile([NLP, TW], BF16)    # local table2 shard pre-allgather

            def gemm_local(stats, mov, ncols, shard, extra):
                for i in range(LT):
                    ps = pg.tile([128, ncols], FP32, tag="ps_g")
                    for k in range(len(mov)):
                        nc.tensor.matmul(ps[:], stats(i, k), mov[k],
                                         start=(k == 0), stop=(k == len(mov) - 1))
                    if shard is not None:
                        stg = stgp.tile([128, ncols], BF16, tag="stg")
                        nc.scalar.copy(stg[:], ps[:])
                        nc.sync.dma_start(shard[i * 128:(i + 1) * 128, 0:ncols], stg[:])
                    if extra is not None:
                        extra(i, ps)

            def stat_xl(i, k):
                return xtl[:, i, k, :]

            # ---- GEMM1 (replicated): table1 = [fc1(x) | el1] for all nodes ----
            for p in range(80):
                ld = statp.tile([128, 2, 2, 128], BF16, tag="ld")
                nc.sync.dma_start(ld[:], xtt_d[p])
                for j in range(2):
                    nt = 2 * p + j
                    ci, li = nt // LT, nt % LT
                    row0 = (li // 5) * (M * 640) + ci * 640 + (li % 5) * 128
                    ps = pg.tile([128, 264], FP32, tag="ps_g")
                    nc.tensor.matmul(ps[:], ld[:, j, 0, :], w1[:, 0, :], start=True, stop=False)
                    nc.tensor.matmul(ps[:], ld[:, j, 1, :], w1[:, 1, :], start=False, stop=True)
                    stg = stgp.tile([128, 264], BF16, tag="stg")
                    nc.vector.tensor_copy(stg[:], ps[:])
                    weng = nc.sync if j == 0 else nc.scalar
                    weng.dma_start(table1[row0:row0 + 128, 0:264], stg[:])

            # ---- local buffer GEMM1: buf1 = x@wb0.T + bb0, er1 local ----
            def keep1(i, ps):
                nc.vector.tensor_tensor(buf1[:, i, :], ps[:, 0:256], bb0r[:],
                                        mybir.AluOpType.add)
                nc.vector.tensor_copy(er1[:, i, :], ps[:, 256:264])
            gemm_local(stat_xl, [wb1[:, 0, :], wb1[:, 1, :]], 264, None, keep1)

            # ---- edge phase ----
            def edge_phase(table, er_loc, heads, drain):
                segw = 256 + heads
                wtiles = {}
                for b in range(nbatch):
                    c0 = b * 8
                    sz = min(8, NCH - c0)
                    ni = sz * 128
                    g = gatp.tile([128, 8, TW], BF16, tag="g")
                    nc.gpsimd.dma_gather(g[:, 0:sz, :], table,
                                         idx_sb[:, c0 * 8:c0 * 8 + sz * 8], ni, ni, TW)
                    oh = ohp.tile([128, 8, 128], BF16, tag="oh")
                    nc.sync.dma_start(oh[:, 0:sz, :],
                                      oh_d[c0:c0 + sz].rearrange("c p f -> p c f"))
                    oht = ohtp.tile([128, 8, 128], BF16, tag="oht")
                    nc.scalar.dma_start(oht[:, 0:sz, :],
                                        oht_d[c0:c0 + sz].rearrange("c p f -> p c f"))
                    # er expansion: per chunk matmul oht^T @ er_w -> [128 e, heads]
                    pse = per.tile([128, 8 * heads], FP32, tag="pse")
                    for c in range(sz):
                        wv = cmap[c0 + c][0]
                        nc.tensor.matmul(pse[:, c * heads:(c + 1) * heads],
                                         oht[:, c, :], er_loc[:, wv, :],
                                         start=True, stop=True)
                    # ex = exp(max(0.2 z, z)), z = el + er
                    z = edp.tile([128, 8, heads], FP32, tag="z")
                    nc.vector.tensor_tensor(
                        z[:, 0:sz, :], g[:, 0:sz, 256:256 + heads],
                        pse[:].rearrange("p (c h) -> p c h", h=heads)[:, 0:sz, :],
                        mybir.AluOpType.add)
                    lr = edp.tile([128, 8, heads], FP32, tag="lr")
                    nc.vector.scalar_tensor_tensor(lr[:, 0:sz, :], z[:, 0:sz, :], 0.2,
                                                   z[:, 0:sz, :],
                                                   mybir.AluOpType.mult, mybir.AluOpType.max)
                    ex = edp.tile([128, 8, heads], BF16, tag="ex")
                    nc.scalar.activation(ex[:, 0:sz, :], lr[:, 0:sz, :],
                                         mybir.ActivationFunctionType.Exp)
                    # scaled = [feat * ex (per head) | ex]
                    sc = edp.tile([128, 8, segw], BF16, tag="sc")
                    if heads == 8:
                        nc.vector.tensor_tensor(
                            sc[:, 0:sz, 0:256].rearrange("p c (h j) -> p c h j", h=8),
                            g[:, 0:sz, 0:256].rearrange("p c (h j) -> p c h j", h=8),
                            ex[:, 0:sz, :].unsqueeze(3).to_broadcast((128, sz, 8, 32)),
                            mybir.AluOpType.mult)
                    else:
                        nc.vector.tensor_tensor(
                            sc[:, 0:sz, 0:256], g[:, 0:sz, 0:256],
                            ex[:, 0:sz, :].to_broadcast((128, sz, 256)),
                            mybir.AluOpType.mult)
                    nc.vector.tensor_copy(sc[:, 0:sz, 256:segw], ex[:, 0:sz, :])
                    # windowed scatter-sum
                    for c in range(sz):
                        wv, first, last = cmap[c0 + c]
                        if first:
                            psw_t = pw.tile([128, segw], FP32, tag="psw")
                            wtiles[wv] = psw_t
                        nc.tensor.matmul(wtiles[wv][:], oh[:, c, :], sc[:, c, :],
                                         start=first, stop=last)
                        if last:
                            drain(wv, wtiles.pop(wv))

            # ---- layer-1 window drain: h1 -> h1T, then GEMM2a tile ----
            def drain1(wv, ps):
                sm = wep.tile([128, 8], FP32, tag="sm")
                nc.vector.tensor_scalar(sm[:], ps[:, 256:264], 1e-30, None,
                                        mybir.AluOpType.max)
                rec = wep.tile([128, 8], FP32, tag="rec")
                nc.vector.reciprocal(rec[:], sm[:])
                rn = wep.tile([128, 256], FP32, tag="rn")
                nc.vector.tensor_tensor(
                    rn[:].rearrange("p (h j) -> p h j", h=8),
                    ps[:, 0:256].rearrange("p (h j) -> p h j", h=8),
                    rec[:].unsqueeze(2).to_broadcast((128, 8, 32)),
                    mybir.AluOpType.mult)
                nc.vector.tensor_tensor(rn[:], rn[:], b1r[:], mybir.AluOpType.add)
                hpre = wep.tile([128, 256], FP32, tag="hpre")
                nc.vector.scalar_tensor_tensor(hpre[:], buf1[:, wv, :], nw[:, wv:wv + 1],
                                               rn[:], mybir.AluOpType.mult,
                                               mybir.AluOpType.add)
                h1b = wep.tile([128, 256], BF16, tag="h1b")
                nc.scalar.activation(h1b[:], hpre[:], mybir.ActivationFunctionType.Relu)
                for fh in range(2):
                    ptt = pt.tile([128, 128], BF16, tag="ptt")
                    nc.tensor.transpose(ptt[:], h1b[:, fh * 128:(fh + 1) * 128], ident[:])
                    nc.vector.tensor_copy(h1T[:, fh, wv * 128:(wv + 1) * 128], ptt[:])
                # GEMM2a for this tile: [fc2(h1) | el2] -> shard2
                ps2 = pg.tile([128, 257], FP32, tag="ps_g")
                for k in range(2):
                    nc.tensor.matmul(ps2[:], h1T[:, k, wv * 128:(wv + 1) * 128],
                                     w2[:, k, :], start=(k == 0), stop=(k == 1))
                stg = stgp.tile([128, 257], BF16, tag="stg")
                nc.scalar.copy(stg[:], ps2[:])
                nc.sync.dma_start(sh2[wv * 128:(wv + 1) * 128, 0:257], stg[:])
                # GEMM2b for this tile: buf2 = cat(x,h1)@wb1.T + bb1, er2 local
                ps3 = pg.tile([128, 257], FP32, tag="ps_g")
                for k in range(4):
                    st = stat_xl(wv, k) if k < 2 else h1T[:, k - 2, wv * 128:(wv + 1) * 128]
                    nc.tensor.matmul(ps3[:], st, wb2[:, k, :],
                                     start=(k == 0), stop=(k == 3))
                nc.vector.tensor_tensor(buf2[:, wv, :], ps3[:, 0:256], bb1r[:],
                                        mybir.AluOpType.add)
                nc.vector.tensor_copy(er2[:, wv, :], ps3[:, 256:257])

            edge_phase(table1[:], er1, 8, drain1)

            # ---- allgather table2 (grouped) ----
            for gg in range(4):
                nc.gpsimd.collective_compute(
                    "AllGather", mybir.AluOpType.bypass,
                    replica_groups=[list(range(M))],
                    ins=[sh2[gg * 640:(gg + 1) * 640, :]], outs=[table2[gg]])

            # ---- layer-2 window drain -> final output ----
            def drain2(wv, ps):
                sm = wep.tile([128, 1], FP32, tag="sm2")
                nc.vector.tensor_scalar(sm[:], ps[:, 256:257], 1e-30, None,
                                        mybir.AluOpType.max)
                rec = wep.tile([128, 1], FP32, tag="rec2")
                nc.vector.reciprocal(rec[:], sm[:])
                rst = wep.tile([128, 256], FP32, tag="rst2")
                nc.vector.tensor_scalar(rst[:], ps[:, 0:256], rec[:], None,
                                        mybir.AluOpType.mult)
                nc.vector.tensor_tensor(rst[:], rst[:], b2r[:], mybir.AluOpType.add)
                ow = wep.tile([128, 256], FP32, tag="ow")
                nc.vector.scalar_tensor_tensor(ow[:], buf2[:, wv, :], nw[:, wv:wv + 1],
                                               rst[:], mybir.AluOpType.mult,
                                               mybir.AluOpType.add)
                nc.sync.dma_start(out_d[wv * 128:(wv + 1) * 128, :], ow[:])

            edge_phase(table2[:].rearrange("g m r w -> (g m r) w"), er2, 1, drain2)

    nc.compile()
    return nc


def _wrap_idx(flat):
    """Per-batch (1024) column-major-of-16 wrap, tiled to 128 partitions."""
    out = []
    for s in range(0, len(flat), 1024):
        blk = flat[s:s + 1024]
        out.append(blk.reshape(-1, 16).T)
    return np.tile(np.concatenate(out, axis=1), (8, 1)).astype(np.int16)


def kernel(**inputs):
    x = np.asarray(inputs["inputs"], np.float32)
    src = np.asarray(inputs["src"], np.int64)
    dst = np.asarray(inputs["dst"], np.int64)

    deg = np.bincount(dst, minlength=N).astype(np.float32)
    norm = 1.0 / np.clip(deg, 1.0, None)
    src_l = src % NL
    src_c = src // NL
    src_n = src_c * NLP + src_l          # padded-global (for nothing but dst calc symmetry)
    dst_n = (dst // NL) * NLP + (dst % NL)
    # table rows are group-major: row(core k, local r) = (r//640)*5120 + k*640 + r%640
    src_t = (src_l // 640) * (M * 640) + src_c * 640 + (src_l % 640)

    # per-core edge lists grouped by window (128 local dst), padded to C chunks
    core_of = dst // NL
    dloc = dst_n % NLP
    win = dloc // 128
    counts = np.zeros((M, LT), np.int64)
    buckets = [[[] for _ in range(LT)] for _ in range(M)]
    order = np.argsort(core_of * LT + win, kind="stable")
    for e in order:
        k, w = core_of[e], win[e]
        buckets[k][w].append(e)
        counts[k, w] += 1
    Cw = tuple(int(np.ceil(counts[:, w].max() / 128)) for w in range(LT))
    NCH = sum(Cw)
    EPAD = NCH * 128
    wbase = np.concatenate([[0], np.cumsum([c * 128 for c in Cw])])

    if Cw not in _cache:
        _cache[Cw] = _build(Cw)
    nc = _cache[Cw]

    # fused weights
    fc1w = np.asarray(inputs["fc1_w"], np.float32)
    al1 = np.asarray(inputs["attn_l1"], np.float32)
    ar1 = np.asarray(inputs["attn_r1"], np.float32)
    Al = np.zeros((256, 8), np.float32); Ar = np.zeros((256, 8), np.float32)
    for h in range(H1):
        Al[h * D1:(h + 1) * D1, h] = al1[h]
        Ar[h * D1:(h + 1) * D1, h] = ar1[h]
    Wl1 = fc1w.T @ Al; Wr1 = fc1w.T @ Ar
    fc2w = np.asarray(inputs["fc2_w"], np.float32)
    Wl2 = fc2w.T @ np.asarray(inputs["attn_l2"], np.float32).T
    Wr2 = fc2w.T @ np.asarray(inputs["attn_r2"], np.float32).T
    w1c = np.concatenate([fc1w.T, Wl1], 1).astype(bf16)              # [256, 264]
    wb1c = np.concatenate([np.asarray(inputs["wb0"], np.float32).T, Wr1], 1).astype(bf16)
    w2c = np.concatenate([fc2w.T, Wl2], 1).astype(bf16)              # [256, 257]
    Wr2p = np.concatenate([np.zeros((256, 1), np.float32), Wr2], 0)
    wb2c = np.concatenate([np.asarray(inputs["wb1"], np.float32).T, Wr2p],
                          1).astype(bf16)                            # [512, 257]

    xp = np.zeros((NP, 256), np.float32)
    for k in range(M):
        xp[k * NLP:k * NLP + NL] = x[k * NL:(k + 1) * NL]
    xt_b = np.ascontiguousarray(xp.T).astype(bf16)                    # [256, NP]
    normp = np.zeros(NP, np.float32)
    for k in range(M):
        normp[k * NLP:k * NLP + NL] = norm[k * NL:(k + 1) * NL]

    xtt = np.zeros((80, 128, 2, 2, 128), bf16)
    for nt in range(160):
        p, j = nt // 2, nt % 2
        ci, li = nt // LT, nt % LT
        cols = slice(ci * NLP + li * 128, ci * NLP + (li + 1) * 128)
        xtt[p, :, j, 0, :] = xt_b[0:128, cols]
        xtt[p, :, j, 1, :] = xt_b[128:256, cols]

    def rep(v):
        return np.broadcast_to(np.asarray(v, np.float32)[None, :], (128, 256)).copy()

    common = {
        "w1": np.ascontiguousarray(w1c.reshape(2, 128, 264).transpose(1, 0, 2)),
        "wb1c": np.ascontiguousarray(wb1c.reshape(2, 128, 264).transpose(1, 0, 2)),
        "w2": np.ascontiguousarray(w2c.reshape(2, 128, 257).transpose(1, 0, 2)),
        "wb2c": np.ascontiguousarray(wb2c.reshape(4, 128, 257).transpose(1, 0, 2)),
        "b1r": rep(inputs["bias1"]), "bb0r": rep(inputs["bb0"]),
        "b2r": rep(inputs["bias2"]), "bb1r": rep(inputs["bb1"]),
        "ident": np.eye(128, dtype=bf16),
    }

    in_maps = []
    for k in range(M):
        gid = np.zeros(EPAD, np.int64)           # gather row ids (pads -> row 0)
        dl = np.zeros(EPAD, np.int64)
        real = np.zeros(EPAD, bool)
        for w in range(LT):
            es = buckets[k][w]
            base = int(wbase[w])
            gid[base:base + len(es)] = src_t[es]
            dl[base:base + len(es)] = dloc[es] - w * 128
            real[base:base + len(es)] = True
        oh = np.zeros((EPAD, 128), bf16)
        oh[np.arange(EPAD)[real], dl[real]] = bf16(1.0)
        oh = oh.reshape(NCH, 128, 128)
        oht = np.ascontiguousarray(oh.transpose(0, 2, 1))
        # local x^T tiles [LT, 2, 128, 128]
        xl = xt_b[:, k * NLP:(k + 1) * NLP]      # [256, 2560]
        xlt = np.ascontiguousarray(
            xl.reshape(2, 128, LT, 128).transpose(2, 0, 1, 3))
        im = dict(common)
        im["idx"] = _wrap_idx(gid)
        im["xtt"] = xtt
        im["oh"] = oh
        im["oht"] = oht
        im["xtl"] = xlt
        im["normw"] = np.ascontiguousarray(
            normp[k * NLP:(k + 1) * NLP].reshape(LT, 128).T)
        in_maps.append(im)

    res = run_bass_kernel_spmd(nc, in_maps, core_ids=list(range(M)), trace=False)
    out = np.empty((N, 256), np.float32)
    for k in range(M):
        out[k * NL:(k + 1) * NL] = res.results[k]["out"][0:NL]
    return out
